# revision 10
# baseline (speedup 1.0000x reference)
"""GATv2 (2-layer) + link-prediction scores on 8 Trainium2 NeuronCores.

Strategy (dst-partitioned, edge-sorted):
  - Nodes padded to 50176 = 392 blocks of 128 rows; core k owns 49 blocks.
  - Edges sorted by dst block; within a block split into A (src < 25088) and
    B (src >= 25088) halves so gather indices fit int16, each half padded to a
    multiple of 128 ("chunks").  Per-block chunk counts are maxed over cores so
    all 8 cores share one compiled program (SPMD).
  - Per layer: project h @ W_src into bf16 tables fsA/fsB (full graph,
    replicated compute) and h_own @ W_dst into fd (own dst rows only); then
    stream edge chunks: bulk-gather fs/fd rows, t = fs+fd, e = lrelu(t),
    logits = reduce(e*attn), ex = exp(logits)  (softmax max-subtraction is
    skipped - logits are O(1)), msg = fs*ex; a selection matrix
    Sel[edge, node] = (localdst == iota) turns segment-sum into PE matmuls
    accumulating [numer | denom] in PSUM per dst block; out = numer/denom.
  - AllGather (8 cores) shares h between layers; queries gather h2 rows and
    reduce dot products; host inverse-permutes the scores.

Numerics: bf16 tables/gathers/matmuls, fp32 logits/exp/denominators.
Verified on host: rel err ~8e-5 vs fp32 reference (tolerance 2e-2).
"""

import os
import numpy as np
import ml_dtypes

BF16 = ml_dtypes.bfloat16

# problem geometry (hardcoded per contract)
N = 50000
E = 800000
Q = 100000
DIM = 128
HEADS = 4
HID = 32
NEG = 0.2

NCORES = 8
NPAD = 50176          # 392 * 128
NBLK_TOT = 392
RPC = NPAD // NCORES  # 6272 rows per core
NBLK = NBLK_TOT // NCORES  # 49 blocks per core
SPLIT = 25088         # = 196*128; idx < 32768 on both halves
QPC = Q // NCORES     # 12500 queries per core
CMAX = 64             # max chunks (128 edges each) per gather group
PAD_LD = 999.0        # localdst for padding edges (never matches iota 0..127)
EPS = 1e-20


# ----------------------------------------------------------------------------
# host-side preprocessing
# ----------------------------------------------------------------------------

def _ceil_div(a, b):
    return -(-a // b)


def preprocess(edge_src, edge_dst, qsrc, qdst):
    """Compute the SPMD-uniform layout + per-core index streams."""
    s = np.asarray(edge_src).astype(np.int64)
    d = np.asarray(edge_dst).astype(np.int64)

    gblk = d >> 7                      # global dst block 0..390
    half = (s >= SPLIT).astype(np.int64)
    key = gblk * 2 + half
    order = np.argsort(key, kind="stable")
    ss, dd = s[order], d[order]
    counts = np.bincount(key, minlength=NBLK_TOT * 2).reshape(NBLK_TOT, 2)
    seg_start = np.zeros(NBLK_TOT * 2, np.int64)
    seg_start[1:] = np.cumsum(counts.reshape(-1))[:-1]
    seg_start = seg_start.reshape(NBLK_TOT, 2)

    nA = counts[:, 0].reshape(NCORES, NBLK)
    nB = counts[:, 1].reshape(NCORES, NBLK)
    CA = _ceil_div(nA, 128).max(axis=0)   # [NBLK] chunks, uniform over cores
    CB = _ceil_div(nB, 128).max(axis=0)

    # greedy-pack consecutive blocks into gather groups of <= CMAX chunks
    groups = []  # (blocks, AC, BC, chunk0)
    cur, cura, curb = [], 0, 0
    chunk0 = 0
    for b in range(NBLK):
        cb = int(CA[b] + CB[b])
        if cur and cura + curb + cb > CMAX:
            groups.append((cur, cura, curb, chunk0))
            chunk0 += cura + curb
            cur, cura, curb = [], 0, 0
        cur.append(b)
        cura += int(CA[b])
        curb += int(CB[b])
    if cur:
        groups.append((cur, cura, curb, chunk0))
        chunk0 += cura + curb
    T = chunk0                       # total chunks per core per layer
    L = T * 128                      # padded edge-stream length

    # per-group layout: [A-parts of blocks (each padded) | B-parts]
    # chunk index of block b's A part / B part inside its group
    a_off = {}
    b_off = {}
    for (blocks, AC, BC, c0) in groups:
        acum, bcum = 0, 0
        for b in blocks:
            a_off[b] = c0 + acum
            b_off[b] = c0 + AC + bcum
            acum += int(CA[b])
            bcum += int(CB[b])

    # per-core streams
    fs_idx = np.zeros((NCORES, L), np.int64)
    fd_idx = np.zeros((NCORES, L), np.int64)
    ld_arr = np.full((NCORES, L), PAD_LD, np.float32)
    for k in range(NCORES):
        for b in range(NBLK):
            gb = k * NBLK + b
            for hlf, off_chunks in ((0, a_off[b]), (1, b_off[b])):
                n = counts[gb, hlf]
                if n == 0:
                    continue
                st = seg_start[gb, hlf]
                dst0 = off_chunks * 128
                seg_s = ss[st:st + n]
                seg_d = dd[st:st + n]
                fs_idx[k, dst0:dst0 + n] = seg_s - (SPLIT if hlf else 0)
                fd_idx[k, dst0:dst0 + n] = seg_d - RPC * k
                ld_arr[k, dst0:dst0 + n] = (seg_d & 127).astype(np.float32)

    # queries: bucket by (qsrc half, qdst half); contiguous QPC per core
    qs = np.asarray(qsrc).astype(np.int64)
    qd = np.asarray(qdst).astype(np.int64)
    bucket = (qs >= SPLIT) * 2 + (qd >= SPLIT)
    qcnt = np.zeros((NCORES, 4), np.int64)
    qorder = []
    for k in range(NCORES):
        sl = slice(k * QPC, (k + 1) * QPC)
        bk = bucket[sl]
        o = np.argsort(bk, kind="stable") + k * QPC
        qorder.append(o)
        qcnt[k] = np.bincount(bk, minlength=4)
    QC = _ceil_div(qcnt, 128).max(axis=0)  # [4]
    TQ = int(QC.sum())
    LQ = TQ * 128
    qb_off = np.zeros(5, np.int64)
    qb_off[1:] = np.cumsum(QC * 128)

    qs_idx = np.zeros((NCORES, LQ), np.int64)
    qd_idx = np.zeros((NCORES, LQ), np.int64)
    qperm = np.full((NCORES, LQ), -1, np.int64)
    for k in range(NCORES):
        o = qorder[k]
        bk = bucket[o]
        pos = 0
        for bb in range(4):
            idxs = o[bk == bb]
            n = len(idxs)
            dst0 = int(qb_off[bb])
            qs_idx[k, dst0:dst0 + n] = qs[idxs] - (SPLIT if bb >= 2 else 0)
            qd_idx[k, dst0:dst0 + n] = qd[idxs] - (SPLIT if (bb & 1) else 0)
            qperm[k, dst0:dst0 + n] = idxs
            pos += n

    def wrap16(a):  # [L] -> [128, L/16] int16; i at [i%16, i//16], x8 rows
        w = np.ascontiguousarray(a.reshape(-1, 16).T).astype(np.int16)
        return np.ascontiguousarray(np.tile(w, (8, 1)))

    layout = dict(groups=groups, CA=CA, CB=CB, a_off=a_off, b_off=b_off,
                  T=T, TQ=TQ, QC=QC)
    percore = []
    for k in range(NCORES):
        percore.append(dict(
            fsidx=wrap16(fs_idx[k]),
            fdidx=wrap16(fd_idx[k]),
            ldbf=np.ascontiguousarray(
                ld_arr[k].reshape(T, 128).T).astype(BF16),
            qsidx=wrap16(qs_idx[k]),
            qdidx=wrap16(qd_idx[k]),
            qperm=qperm[k],
        ))
    return layout, percore


# ----------------------------------------------------------------------------
# device program
# ----------------------------------------------------------------------------

def build_program(layout, enable_asserts=False):
    import concourse.bacc as bacc
    import concourse.tile as tile
    import concourse.mybir as mybir
    from concourse import bass
    from concourse._compat import axon_active

    dt = mybir.dt
    AF = mybir.ActivationFunctionType
    AX = mybir.AxisListType
    OP = mybir.AluOpType

    T = layout["T"]
    TQ = layout["TQ"]
    groups = layout["groups"]
    CA, CB = layout["CA"], layout["CB"]
    a_off, b_off = layout["a_off"], layout["b_off"]
    QC = layout["QC"]

    nc = bacc.Bacc(
        "TRN2",
        target_bir_lowering=False,
        debug=False,
        enable_asserts=enable_asserts,
        num_devices=NCORES,
    )

    def inp(name, shape, dtype):
        return nc.declare_dram_parameter(name, list(shape), dtype,
                                         isOutput=False)

    feat_bf = inp("feat_bf", [NPAD, DIM], dt.bfloat16)
    feat_own = inp("feat_own", [RPC, DIM], dt.bfloat16)
    w1s_d = inp("w1s", [DIM, DIM], dt.bfloat16)
    w1d_d = inp("w1d", [DIM, DIM], dt.bfloat16)
    w2s_d = inp("w2s", [DIM, DIM], dt.bfloat16)
    w2d_d = inp("w2d", [DIM, DIM], dt.bfloat16)
    attn1_d = inp("attn1t", [DIM, DIM], dt.bfloat16)
    attn2_d = inp("attn2t", [DIM, DIM], dt.bfloat16)
    iota_d = inp("iotat", [DIM, DIM], dt.bfloat16)
    fsidx_d = inp("fsidx", [128, T * 8], dt.int16)
    fdidx_d = inp("fdidx", [128, T * 8], dt.int16)
    ldbf_d = inp("ldbf", [DIM, T], dt.bfloat16)
    qsidx_d = inp("qsidx", [128, TQ * 8], dt.int16)
    qdidx_d = inp("qdidx", [128, TQ * 8], dt.int16)

    scores_d = nc.declare_dram_parameter("scores", [DIM, TQ], dt.float32,
                                         isOutput=True)

    # internal DRAM
    fsA = [nc.dram_tensor(f"fs{l}A", [SPLIT, DIM], dt.bfloat16) for l in (1, 2)]
    fsB = [nc.dram_tensor(f"fs{l}B", [SPLIT, DIM], dt.bfloat16) for l in (1, 2)]
    fdt = [nc.dram_tensor(f"fd{l}", [RPC, DIM], dt.bfloat16) for l in (1, 2)]
    h_part = [nc.dram_tensor(f"h{l}_part", [RPC, DIM], dt.bfloat16)
              for l in (1, 2)]
    h_full = [nc.dram_tensor(f"h{l}_full", [NPAD, DIM], dt.bfloat16,
                             addr_space="Shared") for l in (1, 2)]
    h2B = nc.dram_tensor("h2B", [NPAD - SPLIT, DIM], dt.bfloat16)

    RG = [list(range(NCORES))]

    with tile.TileContext(nc) as tc:
        from contextlib import ExitStack
        with ExitStack() as ctx:
            # resident pool: index streams, consts, weights
            rp = ctx.enter_context(tc.tile_pool(name="resident", bufs=1))
            fsidx_sb = rp.tile([128, T * 8], dt.int16, tag="fsidx")
            fdidx_sb = rp.tile([128, T * 8], dt.int16, tag="fdidx")
            ld_sb = rp.tile([DIM, T], dt.bfloat16, tag="ldbf")
            iota_sb = rp.tile([DIM, DIM], dt.bfloat16, tag="iota")
            attn_sb = [rp.tile([DIM, DIM], dt.bfloat16, tag=f"attn{l}",
                               name=f"attn_sb{l}") for l in (1, 2)]
            ws_sb = [rp.tile([DIM, DIM], dt.bfloat16, tag=f"ws{l}",
                             name=f"ws_sb{l}") for l in (1, 2)]
            wd_sb = [rp.tile([DIM, DIM], dt.bfloat16, tag=f"wd{l}",
                             name=f"wd_sb{l}") for l in (1, 2)]
            nc.sync.dma_start(out=fsidx_sb[:], in_=fsidx_d[:])
            nc.sync.dma_start(out=fdidx_sb[:], in_=fdidx_d[:])
            nc.sync.dma_start(out=ld_sb[:], in_=ldbf_d[:])
            nc.sync.dma_start(out=iota_sb[:], in_=iota_d[:])
            nc.sync.dma_start(out=attn_sb[0][:], in_=attn1_d[:])
            nc.sync.dma_start(out=attn_sb[1][:], in_=attn2_d[:])
            nc.sync.dma_start(out=ws_sb[0][:], in_=w1s_d[:])
            nc.sync.dma_start(out=ws_sb[1][:], in_=w2s_d[:])
            nc.sync.dma_start(out=wd_sb[0][:], in_=w1d_d[:])
            nc.sync.dma_start(out=wd_sb[1][:], in_=w2d_d[:])

            pp = ctx.enter_context(tc.tile_pool(name="proj", bufs=3))
            pps = ctx.enter_context(
                tc.tile_pool(name="projpsum", bufs=2, space="PSUM"))
            ep = ctx.enter_context(tc.tile_pool(name="edge", bufs=2))
            sp = ctx.enter_context(tc.tile_pool(name="small", bufs=2))
            bp = ctx.enter_context(tc.tile_pool(name="blk", bufs=3))
            psp = ctx.enter_context(
                tc.tile_pool(name="edgepsum", bufs=2, space="PSUM"))

            for li in range(2):
                h_src = feat_bf if li == 0 else h_full[0].ap()
                h_own = feat_own if li == 0 else h_part[0].ap()
                wS, wD, at = ws_sb[li], wd_sb[li], attn_sb[li]
                fsA_l, fsB_l, fd_l = fsA[li].ap(), fsB[li].ap(), fdt[li].ap()

                # ---- projection: fs tables (full graph) ----
                for jj in range(NPAD // 512):  # 98 groups of 4 blocks
                    ht = pp.tile([DIM, 512], dt.bfloat16, tag="ht")
                    nc.sync.dma_start(
                        out=ht[:], in_=h_src[jj * 512:(jj + 1) * 512, :],
                        transpose=True)
                    st = pp.tile([DIM, 4, DIM], dt.bfloat16, tag="st")
                    for c in range(4):
                        ps = pps.tile([DIM, DIM], dt.float32, tag="pp")
                        nc.tensor.matmul(ps[:], lhsT=ht[:, c * 128:(c + 1) * 128],
                                         rhs=wS[:], start=True, stop=True)
                        nc.vector.tensor_copy(st[:, c, :], ps[:])
                    tgt = fsA_l if jj < 49 else fsB_l
                    r0 = jj * 512 if jj < 49 else (jj - 49) * 512
                    nc.sync.dma_start(
                        out=tgt[r0:r0 + 512, :].rearrange(
                            "(c p) f -> p c f", p=128),
                        in_=st[:])

                # ---- projection: fd table (own rows) ----
                for jj in range(13):  # 12 * 512 + 1 * 128 = 6272
                    nb = 4 if jj < 12 else 1
                    rows = nb * 128
                    ht = pp.tile([DIM, rows], dt.bfloat16, tag="ht")
                    nc.sync.dma_start(
                        out=ht[:], in_=h_own[jj * 512:jj * 512 + rows, :],
                        transpose=True)
                    st = pp.tile([DIM, nb, DIM], dt.bfloat16, tag="st")
                    for c in range(nb):
                        ps = pps.tile([DIM, DIM], dt.float32, tag="pp")
                        nc.tensor.matmul(ps[:], lhsT=ht[:, c * 128:(c + 1) * 128],
                                         rhs=wD[:], start=True, stop=True)
                        nc.vector.tensor_copy(st[:, c, :], ps[:])
                    nc.sync.dma_start(
                        out=fd_l[jj * 512:jj * 512 + rows, :].rearrange(
                            "(c p) f -> p c f", p=128),
                        in_=st[:])

                # ---- edge phase ----
                hp = h_part[li].ap()
                for (blocks, AC, BC, c0) in groups:
                    C = AC + BC
                    if C == 0:
                        continue
                    e0 = c0 * 128
                    fs_t = ep.tile([DIM, C, DIM], dt.bfloat16, tag="fs")
                    fd_t = ep.tile([DIM, C, DIM], dt.bfloat16, tag="fd")
                    if AC > 0:
                        nc.gpsimd.dma_gather(
                            out_ap=fs_t[:, :AC, :], in_ap=fsA_l,
                            idxs_ap=fsidx_sb[:, e0 // 16:(e0 + AC * 128) // 16],
                            num_idxs=AC * 128, num_idxs_reg=AC * 128,
                            elem_size=DIM, queue_num=0,
                            single_packet=False)
                    if BC > 0:
                        nc.gpsimd.dma_gather(
                            out_ap=fs_t[:, AC:, :], in_ap=fsB_l,
                            idxs_ap=fsidx_sb[:, (e0 + AC * 128) // 16:
                                             (e0 + C * 128) // 16],
                            num_idxs=BC * 128, num_idxs_reg=BC * 128,
                            elem_size=DIM, queue_num=0,
                            single_packet=False)
                    nc.gpsimd.dma_gather(
                        out_ap=fd_t[:], in_ap=fd_l,
                        idxs_ap=fdidx_sb[:, e0 // 16:(e0 + C * 128) // 16],
                        num_idxs=C * 128, num_idxs_reg=C * 128,
                        elem_size=DIM, queue_num=0,
                            single_packet=False)

                    # t = fs + fd ; e = lrelu(t) ; em = e * attn
                    nc.vector.tensor_add(fd_t[:], fs_t[:], fd_t[:])
                    nc.scalar.activation(fd_t[:], fd_t[:], AF.Lrelu, alpha=NEG)
                    nc.vector.tensor_tensor(
                        fd_t[:], fd_t[:],
                        at[:].unsqueeze(1).broadcast_to([DIM, C, DIM]),
                        op=OP.mult)
                    lg = sp.tile([DIM, C, HEADS], dt.float32, tag="lg")
                    nc.vector.tensor_reduce(
                        lg[:],
                        fd_t[:].rearrange("p c (h w) -> p c h w", h=HEADS),
                        axis=AX.X, op=OP.add)
                    # msgex = [fs * exp(logits)bcast | exp(logits)]
                    me = ep.tile([DIM, C, DIM + HEADS], dt.bfloat16, tag="me")
                    nc.scalar.activation(me[:, :, DIM:], lg[:], AF.Exp)
                    nc.vector.tensor_tensor(
                        me[:, :, :DIM].rearrange("p c (h w) -> p c h w",
                                                 h=HEADS),
                        fs_t[:].rearrange("p c (h w) -> p c h w", h=HEADS),
                        me[:, :, DIM:].unsqueeze(3).broadcast_to(
                            [DIM, C, HEADS, HID]),
                        op=OP.mult)
                    # Sel[edge, node] = (localdst == iota)
                    sel = ep.tile([DIM, C, DIM], dt.bfloat16, tag="sel")
                    nc.vector.tensor_tensor(
                        sel[:],
                        ld_sb[:, c0:c0 + C].unsqueeze(2).broadcast_to(
                            [DIM, C, DIM]),
                        iota_sb[:].unsqueeze(1).broadcast_to([DIM, C, DIM]),
                        op=OP.is_equal)

                    for b in blocks:
                        cl = ([a_off[b] - c0 + i for i in range(int(CA[b]))] +
                              [b_off[b] - c0 + i for i in range(int(CB[b]))])
                        osb = bp.tile([DIM, DIM], dt.bfloat16, tag="osb")
                        if not cl:
                            nc.vector.memset(osb[:], 0)
                        else:
                            nd = psp.tile([DIM, DIM + HEADS], dt.float32,
                                          tag="nd")
                            for i, c in enumerate(cl):
                                nc.tensor.matmul(
                                    nd[:], lhsT=sel[:, c, :], rhs=me[:, c, :],
                                    start=(i == 0), stop=(i == len(cl) - 1))
                            den = bp.tile([DIM, HEADS], dt.float32, tag="den")
                            rec = bp.tile([DIM, HEADS], dt.float32, tag="rec")
                            nc.vector.tensor_scalar_add(den[:], nd[:, DIM:],
                                                        EPS)
                            nc.vector.reciprocal(rec[:], den[:])
                            nc.vector.tensor_tensor(
                                osb[:].rearrange("p (h w) -> p h w", h=HEADS),
                                nd[:, :DIM].rearrange("p (h w) -> p h w",
                                                      h=HEADS),
                                rec[:].unsqueeze(2).broadcast_to(
                                    [DIM, HEADS, HID]),
                                op=OP.mult)
                            if li == 0:
                                nc.scalar.activation(osb[:], osb[:], AF.Relu)
                        nc.sync.dma_start(out=hp[b * 128:(b + 1) * 128, :],
                                          in_=osb[:])

                # ---- share h across cores ----
                nc.gpsimd.collective_compute(
                    "AllGather", OP.bypass, replica_groups=RG,
                    ins=[h_part[li].ap().opt()],
                    outs=[h_full[li].ap().opt()])

            # ---- queries ----
            nc.gpsimd.dma_start(out=h2B.ap()[:], in_=h_full[1].ap()[SPLIT:, :])
            h2A_l = h_full[1].ap()
            h2B_l = h2B.ap()
            qp = ctx.enter_context(tc.tile_pool(name="query", bufs=1))
            hs_t = qp.tile([DIM, TQ, DIM], dt.bfloat16, tag="hs")
            hd_t = qp.tile([DIM, TQ, DIM], dt.bfloat16, tag="hd")
            qoff = np.zeros(5, np.int64)
            qoff[1:] = np.cumsum(QC)
            # qsrc gathers: buckets 0-1 from A, 2-3 from B
            nA_chunks = int(QC[0] + QC[1])

            def qgather(out_ap, table, idx_sb, chunk_lo, chunk_hi, q):
                n = (chunk_hi - chunk_lo) * 128
                if n <= 0:
                    return
                nc.gpsimd.dma_gather(
                    out_ap=out_ap, in_ap=table,
                    idxs_ap=idx_sb[:, chunk_lo * 8:chunk_hi * 8],
                    num_idxs=n, num_idxs_reg=n, elem_size=DIM, queue_num=0,
                            single_packet=False)

            qsidx_sb = qp.tile([128, TQ * 8], dt.int16, tag="qsidx")
            qdidx_sb = qp.tile([128, TQ * 8], dt.int16, tag="qdidx")
            nc.sync.dma_start(out=qsidx_sb[:], in_=qsidx_d[:])
            nc.sync.dma_start(out=qdidx_sb[:], in_=qdidx_d[:])
            qgather(hs_t[:, :nA_chunks, :], h2A_l, qsidx_sb, 0, nA_chunks, 0)
            qgather(hs_t[:, nA_chunks:, :], h2B_l, qsidx_sb, nA_chunks, TQ, 1)
            qgather(hd_t[:, :int(QC[0]), :], h2A_l, qdidx_sb,
                    0, int(qoff[1]), 2)
            qgather(hd_t[:, int(qoff[1]):int(qoff[2]), :], h2B_l, qdidx_sb,
                    int(qoff[1]), int(qoff[2]), 3)
            qgather(hd_t[:, int(qoff[2]):int(qoff[3]), :], h2A_l, qdidx_sb,
                    int(qoff[2]), int(qoff[3]), 0)
            qgather(hd_t[:, int(qoff[3]):TQ, :], h2B_l, qdidx_sb,
                    int(qoff[3]), TQ, 1)

            nc.vector.tensor_tensor(hs_t[:], hs_t[:], hd_t[:], op=OP.mult)
            sc_t = qp.tile([DIM, TQ], dt.float32, tag="sc")
            nc.vector.tensor_reduce(sc_t[:], hs_t[:], axis=AX.X, op=OP.add)
            out_t = qp.tile([DIM, TQ], dt.float32, tag="out")
            nc.scalar.activation(out_t[:], sc_t[:], AF.Sigmoid)
            nc.sync.dma_start(out=scores_d[:], in_=out_t[:])

    nc.compile()
    return nc


# ----------------------------------------------------------------------------
# driver
# ----------------------------------------------------------------------------

def make_in_maps(inputs, layout, percore):
    feat = np.asarray(inputs["feat"], np.float32)
    featp = np.zeros((NPAD, DIM), np.float32)
    featp[:N] = feat
    feat_bf = featp.astype(BF16)
    w1s = np.asarray(inputs["W1_src"], np.float32).astype(BF16)
    w1d = np.asarray(inputs["W1_dst"], np.float32).astype(BF16)
    w2s = np.asarray(inputs["W2_src"], np.float32).astype(BF16)
    w2d = np.asarray(inputs["W2_dst"], np.float32).astype(BF16)
    at1 = np.asarray(inputs["attn1"], np.float32).reshape(-1)
    at2 = np.asarray(inputs["attn2"], np.float32).reshape(-1)
    attn1t = np.broadcast_to(at1, (DIM, DIM)).astype(BF16).copy()
    attn2t = np.broadcast_to(at2, (DIM, DIM)).astype(BF16).copy()
    iotat = np.broadcast_to(np.arange(DIM, dtype=np.float32),
                            (DIM, DIM)).astype(BF16).copy()
    in_maps = []
    for k in range(NCORES):
        pc = percore[k]
        in_maps.append({
            "feat_bf": feat_bf,
            "feat_own": feat_bf[k * RPC:(k + 1) * RPC].copy(),
            "w1s": w1s, "w1d": w1d, "w2s": w2s, "w2d": w2d,
            "attn1t": attn1t, "attn2t": attn2t, "iotat": iotat,
            "fsidx": pc["fsidx"], "fdidx": pc["fdidx"], "ldbf": pc["ldbf"],
            "qsidx": pc["qsidx"], "qdidx": pc["qdidx"],
        })
    return in_maps


def assemble_scores(results, percore):
    out = np.zeros(Q, np.float32)
    for k in range(NCORES):
        sc = results[k]["scores"]          # [128, TQ]
        flat = np.ascontiguousarray(sc.T).reshape(-1)  # query (c,p) -> c*128+p
        perm = percore[k]["qperm"]
        m = perm >= 0
        out[perm[m]] = flat[m]
    return out


_CACHE = {}


def kernel(**inputs):
    inputs = {k: np.asarray(v) for k, v in inputs.items()}
    layout, percore = preprocess(inputs["edge_src"], inputs["edge_dst"],
                                 inputs["qsrc"], inputs["qdst"])
    key = (layout["T"], layout["TQ"])
    if key not in _CACHE:
        _CACHE[key] = build_program(layout)
    nc = _CACHE[key]
    in_maps = make_in_maps(inputs, layout, percore)

    if os.environ.get("GAT_SIM") == "1":
        from concourse.bass_interp import MultiCoreSim
        _patch_sim_lrelu()
        sim = MultiCoreSim(nc, num_cores=NCORES, require_finite=False,
                           require_nnan=False)
        for k in range(NCORES):
            for name, arr in in_maps[k].items():
                sim.cores[k].tensor(name)[:] = arr
        sim.simulate(check_with_hw=False)
        results = [{"scores": np.asarray(sim.cores[k].mem_tensor("scores"))}
                   for k in range(NCORES)]
    else:
        from concourse.bass_utils import run_bass_kernel_spmd
        trace = os.environ.get("GAT_TRACE") == "1"
        if trace:
            _install_ntff_hook()
        res = run_bass_kernel_spmd(nc, in_maps, list(range(NCORES)),
                                   trace=trace)
        if trace and res.exec_time_ns is not None:
            print("HW exec time:", int(res.exec_time_ns), "ns")
        kernel.last_results = res
        results = res.results
    return assemble_scores(results, percore)


def _patch_sim_lrelu():
    """CoreSim doesn't implement Lrelu; emulate via Identity + postprocess."""
    import concourse.mybir as mb
    from concourse import bass_interp
    ex = bass_interp.InstructionExecutor
    if getattr(ex, "_lrelu_patched", False):
        return
    orig = ex.visit_InstActivation

    def patched(self, instruction, *, reg_snapshot=None):
        if instruction.func == mb.ActivationFunctionType.Lrelu:
            alpha = 0.2
            if len(instruction.ins) > 3 and hasattr(instruction.ins[3],
                                                    "value"):
                alpha = float(instruction.ins[3].value)
            instruction.func = mb.ActivationFunctionType.Identity
            try:
                orig(self, instruction, reg_snapshot=reg_snapshot)
            finally:
                instruction.func = mb.ActivationFunctionType.Lrelu
            from concourse.bass_interp import Direction
            out_view = self.view_ap(instruction.outs[0], Direction.WRITE,
                                    instruction, reg_snapshot=reg_snapshot)
            x = out_view[:].astype(np.float32)
            out_view[:] = np.where(x > 0, x, alpha * x).astype(out_view.dtype)
            return
        return orig(self, instruction, reg_snapshot=reg_snapshot)

    ex.visit_InstActivation = patched
    ex._lrelu_patched = True


def _install_ntff_hook(so_path="/opt/axon/libaxon_pjrt.so"):
    """Provide antenv.axon_hooks (missing in this image) so that
    run_bass_kernel_spmd(trace=True) can capture an NTFF profile."""
    import sys, types, ctypes, contextlib
    try:
        from antenv.axon_hooks import get_axon_ntff_profile_hook  # noqa: F401
        return
    except ImportError:
        pass
    lib = ctypes.CDLL(so_path)
    if not hasattr(lib, "axon_start_nrt_profile"):
        return
    lib.axon_start_nrt_profile.argtypes = [ctypes.POINTER(ctypes.c_int64),
                                           ctypes.c_size_t]
    lib.axon_start_nrt_profile.restype = ctypes.c_int64
    lib.axon_stop_nrt_profile.argtypes = [ctypes.c_char_p]
    lib.axon_stop_nrt_profile.restype = ctypes.c_int64

    @contextlib.contextmanager
    def _hook(output_dir, device_ids):
        import jax
        jax.devices()
        if device_ids:
            ids = (ctypes.c_int64 * len(device_ids))(*device_ids)
            rc = lib.axon_start_nrt_profile(ids, len(device_ids))
        else:
            rc = lib.axon_start_nrt_profile(None, 0)
        if rc != 0:
            raise RuntimeError(f"axon_start_nrt_profile rc={rc}")
        try:
            yield
        finally:
            n = lib.axon_stop_nrt_profile(str(output_dir).encode())
            print(f"ntff profile: {n} file(s) -> {output_dir}")

    mod = types.ModuleType("antenv.axon_hooks")
    mod.get_axon_ntff_profile_hook = lambda: _hook
    mod.set_axon_ntff_profile_hook = lambda h: None
    sys.modules["antenv.axon_hooks"] = mod
    import antenv
    antenv.axon_hooks = mod
